# revision 4
# baseline (speedup 1.0000x reference)
"""ComplexMultiheadAttention on TRN2 NeuronCores — transfer-optimized.

The axon tunnel to the remote cores moves ~50 MB/s, so the metric
(wall-time of one cached spmd call) is dominated by bytes shipped, not
by on-device compute (~26 GFLOP total).  Strategy:

 - batch-parallel over 2 cores only (B=2): each core runs the FULL
   pipeline (complex QKV proj, 16-head attention, complex out-proj) for
   one batch.  No cross-core partial sums, no duplicated x.
 - everything shipped in fp16 (tolerance is 2e-2; fp16 end-to-end lands
   ~1e-3).  Weights shipped WITHOUT the [-wi|wr] complex duplication --
   the negated halves are built on device.  Per-core input: 24 MB
   (vs 96 MB for the old 8-core f32 layout); output 8 MB (vs 64 MB).
 - x ships in natural [T, D] layout; device transposes via DMA-xbar.

Device layout tricks (per core, T=2048 tokens, 16 heads, d=64):
 - complex packing: K/M dims carry [real(64)|imag(64)] stacked to 128
 - scores computed transposed (S^T[j,i]) so exp'd probs feed the PV
   matmul directly as rhs with lhsT = V^T tiles -- no transposes
 - softmax denominators via a ones[128,1] lhsT matmul accumulated over
   j-tiles; normalization + v-bias fused into the OT epilogue
 - exp computed as exp(s/8 - 4): the -4 cancels in normalization and
   keeps fp16 prob magnitudes centered
"""
import os
import numpy as np

from concourse import bacc
import concourse.mybir as mybir
import concourse.tile as tile
from concourse.bass_utils import run_bass_kernel_spmd

B, T, D, H = 2, 2048, 1024, 16
d = D // H          # 64
NDT = D // 128      # 8  k-tiles over model dim
NIC = T // 512      # 4  i-chunks (query)
NJT = T // 128      # 16 j-tiles (key)
NET = D // 128      # 8  e-tiles (out-proj output dim)
NG = 4              # head groups
GH = H // NG        # 4 heads per group

F16 = mybir.dt.float16
F32 = mybir.dt.float32
AF = mybir.ActivationFunctionType
NPF16 = np.float16

_PROG = None


def _build_program():
    nc = bacc.Bacc()
    xr = nc.dram_tensor("xr", [T, D], F16, kind="ExternalInput")
    xi = nc.dram_tensor("xi", [T, D], F16, kind="ExternalInput")
    # [wr_h | wi_h] packed per head / per 128-row k-tile (positive only)
    aq = nc.dram_tensor("aq", [H, NDT, 128, 128], F16, kind="ExternalInput")
    ak = nc.dram_tensor("ak", [H, NDT, 128, 128], F16, kind="ExternalInput")
    # per k-tile: cols = per head [vr_h(64) | vi_h(64)]
    av = nc.dram_tensor("av", [NDT, 128, H * 128], F16, kind="ExternalInput")
    # per head / per out e-tile: rows = [o_wr[ch] (64) ; o_wi[ch] (64)]
    ao = nc.dram_tensor("ao", [H, NET, 128, 128], F16, kind="ExternalInput")
    qb = nc.dram_tensor("qb", [128, H], F32, kind="ExternalInput")
    kb = nc.dram_tensor("kb", [128, H], F32, kind="ExternalInput")
    vb = nc.dram_tensor("vb", [128, H], F32, kind="ExternalInput")
    ob = nc.dram_tensor("ob", [128, 2, NET], F32, kind="ExternalInput")
    ones = nc.dram_tensor("ones", [128, 1], F16, kind="ExternalInput")
    yt = nc.dram_tensor("yt", [2, D, T], F16, kind="ExternalOutput")

    with tile.TileContext(nc) as tc:
        with tc.tile_pool(name="bias", bufs=1) as biasp, \
             tc.tile_pool(name="store", bufs=1) as store:
            qb_sb = biasp.tile([128, H], F32, tag="qb")
            kb_sb = biasp.tile([128, H], F32, tag="kb")
            vb_sb = biasp.tile([128, H], F32, tag="vb")
            ob_sb = biasp.tile([128, 2, NET], F32, tag="ob")
            ones_sb = biasp.tile([128, 1], F16, tag="ones")
            negf_sb = biasp.tile([128, 1], F32, tag="negf")
            nc.vector.memset(negf_sb[:], -4.0)
            nc.sync.dma_start(qb_sb[:], qb[:])
            nc.sync.dma_start(kb_sb[:], kb[:])
            nc.sync.dma_start(vb_sb[:], vb[:])
            nc.sync.dma_start(ob_sb[:], ob[:])
            nc.sync.dma_start(ones_sb[:], ones[:])

            # attention outputs, all 16 heads: rows [or(64)|oi(64)]
            OT = [store.tile([128, T], F16, tag=f"ot{h}", name=f"ot{h}")
                  for h in range(H)]

            with tc.tile_pool(name="xp", bufs=1) as xp:
                xr_sb = xp.tile([128, NDT, T], F16, tag="xr")
                xi_sb = xp.tile([128, NDT, T], F16, tag="xi")
                for dt in range(NDT):
                    dsl = slice(dt * 128, (dt + 1) * 128)
                    nc.sync.dma_start_transpose(xr_sb[:, dt, :], xr[:, dsl])
                    nc.sync.dma_start_transpose(xi_sb[:, dt, :], xi[:, dsl])

                for g in range(NG):
                    heads = list(range(GH * g, GH * (g + 1)))
                    with tc.tile_pool(name="grp", bufs=1) as grp:
                        QT = [grp.tile([128, T], F16, tag=f"qt{i}", name=f"qt{i}")
                              for i in range(GH)]
                        KT = [grp.tile([128, T], F16, tag=f"kt{i}", name=f"kt{i}")
                              for i in range(GH)]
                        # V^T per j-tile: cols = per group head [vr|vi]
                        VS = [grp.tile([128, GH * 128], F16, tag=f"vs{jt}", name=f"vs{jt}")
                              for jt in range(NJT)]

                        # ---- Q then K projection (x resident in SBUF) ----
                        with tc.tile_pool(name="wqk", bufs=1) as wqk, \
                             tc.tile_pool(name="psqk", bufs=1, space="PSUM") as psqk:
                            for aw, bias_sb, DT in ((aq, qb_sb, QT), (ak, kb_sb, KT)):
                                aw_sb = wqk.tile([128, GH, NDT, 128], F16, tag="aw", name="aw")
                                awn_sb = wqk.tile([128, GH, NDT, 128], F16, tag="awn", name="awn")
                                for i, h in enumerate(heads):
                                    for dt in range(NDT):
                                        nc.sync.dma_start(aw_sb[:, i, dt, :], aw[h, dt])
                                for i in range(GH):
                                    nc.scalar.mul(awn_sb[:, i, :, 0:64],
                                                  aw_sb[:, i, :, 64:128], -1.0)
                                    nc.scalar.copy(awn_sb[:, i, :, 64:128],
                                                   aw_sb[:, i, :, 0:64])
                                for ic in range(NIC):
                                    icsl = slice(ic * 512, (ic + 1) * 512)
                                    ps = [psqk.tile([128, 512], F32, tag=f"ps{i}", name=f"ps{i}")
                                          for i in range(GH)]
                                    for dt in range(NDT):
                                        st = (dt == 0)
                                        sp = (dt == NDT - 1)
                                        for i in range(GH):
                                            nc.tensor.matmul(ps[i][:], aw_sb[:, i, dt, :],
                                                             xr_sb[:, dt, icsl],
                                                             start=st, stop=False)
                                            nc.tensor.matmul(ps[i][:], awn_sb[:, i, dt, :],
                                                             xi_sb[:, dt, icsl],
                                                             start=False, stop=sp)
                                    for i, h in enumerate(heads):
                                        nc.scalar.activation(DT[i][:, icsl], ps[i][:],
                                                             AF.Identity,
                                                             bias=bias_sb[:, h:h + 1])

                        # ---- V projection ----
                        with tc.tile_pool(name="wv", bufs=1) as wv, \
                             tc.tile_pool(name="psv", bufs=2, space="PSUM") as psv:
                            av_sb = wv.tile([128, NDT, 512], F16, tag="av")
                            avn_sb = wv.tile([128, NDT, 512], F16, tag="avn")
                            for dt in range(NDT):
                                nc.sync.dma_start(av_sb[:, dt, :],
                                                  av[dt, :, g * 512:(g + 1) * 512])
                            for i in range(GH):
                                nc.scalar.mul(avn_sb[:, :, i * 128:i * 128 + 64],
                                              av_sb[:, :, i * 128 + 64:i * 128 + 128], -1.0)
                                nc.scalar.copy(avn_sb[:, :, i * 128 + 64:i * 128 + 128],
                                               av_sb[:, :, i * 128:i * 128 + 64])
                            for ic in range(NIC):
                                pv = [psv.tile([128, 512], F32, tag=f"pv{jj}", name=f"pv{jj}")
                                      for jj in range(4)]
                                for dt in range(NDT):
                                    for jj in range(4):
                                        jsl = slice(ic * 512 + jj * 128,
                                                    ic * 512 + (jj + 1) * 128)
                                        nc.tensor.matmul(pv[jj][:], xr_sb[:, dt, jsl],
                                                         av_sb[:, dt, :],
                                                         start=(dt == 0), stop=False)
                                        nc.tensor.matmul(pv[jj][:], xi_sb[:, dt, jsl],
                                                         avn_sb[:, dt, :],
                                                         start=False, stop=(dt == NDT - 1))
                                for jj in range(4):
                                    nc.scalar.copy(VS[ic * 4 + jj][:], pv[jj][:])

                        # ---- attention ----
                        with tc.tile_pool(name="pexp", bufs=4) as pexp, \
                             tc.tile_pool(name="pnorm", bufs=2) as pnorm, \
                             tc.tile_pool(name="pss", bufs=2, space="PSUM") as pss, \
                             tc.tile_pool(name="pso", bufs=2, space="PSUM") as pso:
                            for i, h in enumerate(heads):
                                for ic in range(NIC):
                                    icsl = slice(ic * 512, (ic + 1) * 512)
                                    ps_o = pso.tile([128, 512], F32, tag="po", name="po")
                                    ps_d = pso.tile([1, 512], F32, tag="pd", name="pd")
                                    for jt in range(NJT):
                                        jsl = slice(jt * 128, (jt + 1) * 128)
                                        ps_s = pss.tile([128, 512], F32, tag="s", name="s")
                                        nc.tensor.matmul(ps_s[:], KT[i][:, jsl],
                                                         QT[i][:, icsl],
                                                         start=True, stop=True)
                                        pt = pexp.tile([128, 512], F16, tag="pt", name="pt")
                                        nc.scalar.activation(pt[:], ps_s[:], AF.Exp,
                                                             scale=0.125,
                                                             bias=negf_sb[:, 0:1])
                                        nc.tensor.matmul(ps_o[:],
                                                         VS[jt][:, i * 128:(i + 1) * 128],
                                                         pt[:],
                                                         start=(jt == 0), stop=(jt == NJT - 1))
                                        nc.tensor.matmul(ps_d[:], ones_sb[:], pt[:],
                                                         start=(jt == 0), stop=(jt == NJT - 1))
                                    recip = pnorm.tile([1, 512], F32, tag="recip", name="recip")
                                    nc.vector.reciprocal(recip[:], ps_d[0:1, :])
                                    rbc = pnorm.tile([128, 512], F32, tag="rbc", name="rbc")
                                    nc.gpsimd.partition_broadcast(rbc[:], recip[:],
                                                                  channels=128)
                                    tmp = pnorm.tile([128, 512], F32, tag="tmp", name="tmp")
                                    nc.vector.tensor_mul(tmp[:], ps_o[:], rbc[:])
                                    nc.scalar.activation(OT[h][:, icsl], tmp[:],
                                                         AF.Identity,
                                                         bias=vb_sb[:, h:h + 1])

            # ---- out projection (x freed; OT for all heads resident) ----
            with tc.tile_pool(name="wo", bufs=1) as wo, \
                 tc.tile_pool(name="ys", bufs=4) as ys, \
                 tc.tile_pool(name="psy", bufs=2, space="PSUM") as psy:
                aoR = wo.tile([128, H, NET, 128], F16, tag="aoR")
                aoI = wo.tile([128, H, NET, 128], F16, tag="aoI")
                for h in range(H):
                    src_r = ao[h, :, 0:64, :].rearrange("n p k -> p n k")
                    src_i = ao[h, :, 64:128, :].rearrange("n p k -> p n k")
                    nc.sync.dma_start(aoR[0:64, h], src_r)
                    nc.sync.dma_start(aoR[64:128, h], src_i)
                    nc.sync.dma_start(aoI[0:64, h], src_i)
                    nc.sync.dma_start(aoI[64:128, h], src_r)
                    # yr lhsT rows 64:128 must hold -o_wi
                    nc.scalar.mul(aoR[64:128, h], aoR[64:128, h], -1.0)
                for et in range(NET):
                    esl = slice(et * 128, (et + 1) * 128)
                    for ic in range(NIC):
                        icsl = slice(ic * 512, (ic + 1) * 512)
                        ps_yr = psy.tile([128, 512], F32, tag="yr", name="yr")
                        ps_yi = psy.tile([128, 512], F32, tag="yi", name="yi")
                        for h in range(H):
                            nc.tensor.matmul(ps_yr[:], aoR[:, h, et, :], OT[h][:, icsl],
                                             start=(h == 0), stop=(h == H - 1))
                            nc.tensor.matmul(ps_yi[:], aoI[:, h, et, :], OT[h][:, icsl],
                                             start=(h == 0), stop=(h == H - 1))
                        ytr = ys.tile([128, 512], F16, tag="ytr", name="ytr")
                        yti = ys.tile([128, 512], F16, tag="yti", name="yti")
                        nc.scalar.activation(ytr[:], ps_yr[:], AF.Identity,
                                             bias=ob_sb[:, 0, et:et + 1])
                        nc.scalar.activation(yti[:], ps_yi[:], AF.Identity,
                                             bias=ob_sb[:, 1, et:et + 1])
                        nc.sync.dma_start(yt[0, esl, icsl], ytr[:])
                        nc.sync.dma_start(yt[1, esl, icsl], yti[:])

    nc.finalize()
    return nc


def _pack_weights(inp):
    """Shared (batch-independent) input tensors, all fp16 pure permutations."""
    def qk_pack(wr, wi):
        a = np.empty((H, NDT, 128, 128), NPF16)
        a[..., 0:64] = wr.reshape(NDT, 128, H, d).transpose(2, 0, 1, 3)
        a[..., 64:128] = wi.reshape(NDT, 128, H, d).transpose(2, 0, 1, 3)
        return a

    av = np.empty((NDT, 128, H, 128), NPF16)
    av[..., 0:64] = inp["v_wr"].reshape(NDT, 128, H, d)
    av[..., 64:128] = inp["v_wi"].reshape(NDT, 128, H, d)

    ao = np.empty((H, NET, 128, 128), NPF16)
    ao[:, :, 0:64, :] = inp["o_wr"].reshape(H, d, NET, 128).transpose(0, 2, 1, 3)
    ao[:, :, 64:128, :] = inp["o_wi"].reshape(H, d, NET, 128).transpose(0, 2, 1, 3)

    def bias2(br, bi):
        out = np.empty((128, H), np.float32)
        out[0:64] = br.reshape(H, d).T
        out[64:128] = bi.reshape(H, d).T
        return out

    ob = np.empty((128, 2, NET), np.float32)
    ob[:, 0, :] = inp["o_br"].reshape(NET, 128).T
    ob[:, 1, :] = inp["o_bi"].reshape(NET, 128).T

    return {
        "aq": qk_pack(inp["q_wr"], inp["q_wi"]),
        "ak": qk_pack(inp["k_wr"], inp["k_wi"]),
        "av": av.reshape(NDT, 128, H * 128),
        "ao": ao,
        "qb": bias2(inp["q_br"], inp["q_bi"]),
        "kb": bias2(inp["k_br"], inp["k_bi"]),
        "vb": bias2(inp["v_br"], inp["v_bi"]),
        "ob": ob,
        "ones": np.ones((128, 1), NPF16),
    }


def kernel(**inputs):
    global _PROG
    inp = {k: np.asarray(v, np.float32) for k, v in inputs.items()}
    if _PROG is None:
        _PROG = _build_program()
    wpack = _pack_weights(inp)
    in_maps = []
    for b in range(B):
        m = dict(wpack)
        m["xr"] = inp["x_real"][b].astype(NPF16)
        m["xi"] = inp["x_imag"][b].astype(NPF16)
        in_maps.append(m)
    trace = os.environ.get("KBENCH_TRACE") == "1"
    import time as _time
    t0 = _time.time()
    res = run_bass_kernel_spmd(_PROG, in_maps, core_ids=list(range(B)),
                               trace=trace)
    kernel.last_run_wall_ns = int((_time.time() - t0) * 1e9)
    if trace:
        kernel.last_exec_time_ns = res.exec_time_ns
        kernel.last_trace = res.instructions_and_trace
    y = np.empty((2, B, T, D), np.float32)
    for c in range(B):
        ytc = np.asarray(res.results[c]["yt"]).astype(np.float32)  # [2, D, T]
        y[0, c] = ytc[0].T
        y[1, c] = ytc[1].T
    return y
